# revision 46
# baseline (speedup 1.0000x reference)
"""Trainium2 Bass kernel: CQT (constant-Q transform) of 2^23 audio samples.

Reference math (jax):
    frames[f, n] = x[f*HOP + n]                  HOP=512, fftLen=2048
    four_r = frames @ wcos.T ; four_i = frames @ wsin.T
    cqt_r  = kr @ four_r - ki @ four_i
    cqt_i  = kr @ four_i + ki @ four_r
    out    = sqrt(cqt_r**2 + cqt_i**2)           # [1, 84, n_frames]

Folded on the host (exact algebra, tiny matrices):
    A = kr@wcos - ki@wsin,  B = kr@wsin + ki@wcos      (each [84, 2048])
    out = sqrt((A @ frames.T)**2 + (B @ frames.T)**2)

Device strategy (8-way shard along the frame axis; kernels replicated):
  - 2048 frames per core.  The bf16 x-shard is laid out host-side so that
    the matmul's moving operand is always a CONTIGUOUS column range: with
    xt[p, c] = x[c*128 + p], contraction chunk kc = 4a + r of frame f needs
    column 4*(f+a) + r, so columns are stored deinterleaved by (frame-block,
    r-plane).  A.T/B.T chunks ride the same DRAM tensor.  (A strided rhs AP
    halves the PE's bf16 stream rate - measured 452 -> 216 ns per matmul.)
  - input DMA is split per frame-block so fb0's matmuls start ~3us in;
    4 fb x 16 kc x {A,B} matmuls accumulate into 8 PSUM banks; a^2+b^2 on
    VectorE; one SWDGE DMA out.  sqrt on the host (monotone, exact).
  - a post-pass splits multi-wait instructions: this walrus build encodes
    at most ONE semaphore wait per instruction.
"""

import sys

if "/opt/trn_rl_repo" not in sys.path:
    sys.path.insert(0, "/opt/trn_rl_repo")

import numpy as np
import ml_dtypes

HOP = 512
FFTLEN = 2048
N_BINS = 84
T_SAMPLES = 8388608
N_FRAMES = (T_SAMPLES - FFTLEN) // HOP + 1  # 16381
N_CORES = 8
F_PER_CORE = 2048                 # frames computed per core (3 junk at the end)
X_COLS_TOTAL = 8204               # sample columns actually needed per core
SHARD_LEN = X_COLS_TOTAL * 128    # 1050112 samples per core
CORE_STRIDE = F_PER_CORE * HOP    # 1048576 samples between shard starts
N_KC = FFTLEN // 128              # 16 contraction chunks
N_FB = F_PER_CORE // 512          # 4 frame blocks of 512 frames
PLANE_COLS = 515                  # columns per r-plane per frame block
FB_COLS = 4 * PLANE_COLS          # 2060
AB_R_COLS = 4 * 2 * N_BINS        # 672: the 4 kc-chunks of A.T/B.T for one r
CH_COLS = AB_R_COLS + PLANE_COLS  # 1187: one [AB_r | fb0 plane r] chunk
FB1_LO = 4 * CH_COLS              # 4748: start of the fb1..fb3 blocks
AB_COLS = N_KC * 2 * N_BINS       # 2688 columns holding A.T/B.T chunks
EXT_COLS = FB1_LO + (N_FB - 1) * FB_COLS  # 10928
N_FC = F_PER_CORE // 128          # 16 output frame chunks (128 frames each)

_PROGRAM = None


def _thin_pe_incs(nc, mybir):
    """Matmuls complete in pc order, so only each accumulation group's last
    matmul needs its PE-semaphore increment.  The PE sequencer retires incs
    at ~115ns each - 267 of them add ~6us of pure sem-retirement tail.
    Strip non-stop matmul incs and renumber every wait on that semaphore."""
    sem_id = None
    tick = 0
    kept = 0
    tick_to_kept = {0: 0}
    for f in nc.m.functions:
        for blk in f.blocks:
            for inst in blk.instructions:
                si = getattr(inst, "sync_info", None)
                if si is None:
                    continue
                pe_ups = [u for u in si.on_update
                          if u.ant_name.startswith("PE")]
                if not pe_ups:
                    continue
                if type(inst).__name__ != "InstMatmult":
                    return  # unexpected PE-sem producer; skip optimization
                sem_id = pe_ups[0].id
                tick += 1
                if inst.stop_tensor_calc:
                    kept += 1
                else:
                    inst.sync_info = mybir.SyncInfo(
                        on_wait=list(si.on_wait),
                        on_update=[u for u in si.on_update
                                   if not u.ant_name.startswith("PE")])
                tick_to_kept[tick] = kept
    if sem_id is None:
        return
    for f in nc.m.functions:
        for blk in f.blocks:
            for inst in blk.instructions:
                si = getattr(inst, "sync_info", None)
                if si is None:
                    continue
                changed = False
                new_waits = []
                for w in si.on_wait:
                    if w.id == sem_id and w.wait_value in tick_to_kept:
                        nv = tick_to_kept[w.wait_value]
                        if nv != w.wait_value:
                            w = mybir.SyncWait(
                                sync_type=w.sync_type, id=w.id,
                                ant_name=w.ant_name, wait_mode=w.wait_mode,
                                wait_value=nv, wait_reg=w.wait_reg)
                            changed = True
                    new_waits.append(w)
                if changed:
                    inst.sync_info = mybir.SyncInfo(
                        on_wait=new_waits, on_update=list(si.on_update))


def _split_multi_waits(nc, mybir, max_waits=1):
    """This walrus build encodes at most one sem wait per instruction; move
    extra waits onto injected same-engine NoOps right before the instruction."""
    ctr = 0
    for f in nc.m.functions:
        for blk in f.blocks:
            il = list(blk.instructions)
            new = []
            changed = False
            for inst in il:
                si = getattr(inst, "sync_info", None)
                if si is not None and len(si.on_wait) > max_waits:
                    waits = list(si.on_wait)
                    for w in waits[:-max_waits]:
                        nop = mybir.InstNoOp(name=f"I-waitfix-{ctr}", ins=[], outs=[])
                        ctr += 1
                        nop.engine = inst.engine
                        nop.sync_info = mybir.SyncInfo(on_wait=[w], on_update=[])
                        new.append(nop)
                    inst.sync_info = mybir.SyncInfo(
                        on_wait=waits[-max_waits:], on_update=list(si.on_update))
                    changed = True
                new.append(inst)
            if changed:
                blk.instructions = new


def _build_program():
    import concourse.bass as bass
    import concourse.tile as tile
    from concourse import mybir
    from concourse.vector_clock import ScopedClock

    def _lean_drain(self, tick_clock, wait_clock):
        # Tail for a single-shot NEFF: the SP drain already waits on every
        # proc's final tick (incl. output-DMA completion).  The stock
        # drain+barrier+sem-reset+barrier tail costs ~7us and only matters
        # for re-executing a loaded NEFF with dirty semaphores.
        drain_inst = self.nc.sync.drain()
        wait_clock.add_sem_waits(
            drain_inst.ins, ScopedClock({None: tick_clock.global_clock}))
        popped = self.nc._tile_sem_poison_stack.pop()
        assert popped is self._sem_poison

    tile.TileContext._drain_and_barrier = _lean_drain

    # Skip the ~3.4us entry all-engine barrier: it orders the preamble's
    # const-AP writes (PE, t~0.4us) and SWDGE scratch memsets against the
    # body.  This kernel reads const APs first at ~13us (ACT square bias)
    # and issues no SWDGE DMAs, so engine start-skew cannot race it.
    _orig_barrier = bass.Bass.all_engine_barrier
    bass.Bass.all_engine_barrier = lambda self, **kw: None
    try:
        nc = bass.Bass("TRN2", target_bir_lowering=False, debug=False)
    finally:
        bass.Bass.all_engine_barrier = _orig_barrier

    ext = nc.dram_tensor("ext", [128, EXT_COLS], mybir.dt.bfloat16,
                         kind="ExternalInput").ap()
    # out[p, fc*84+j] = |cqt|^2 at frame fc*128+p, bin j
    out = nc.dram_tensor("out", [128, N_FC * N_BINS], mybir.dt.float32,
                         kind="ExternalOutput").ap()

    with tile.TileContext(nc) as tc:
        with (
            tc.tile_pool(name="const", bufs=1) as const,
            tc.tile_pool(name="psum", bufs=4, space="PSUM") as psum,
            tc.tile_pool(name="tmp", bufs=4) as tmp,
            tc.tile_pool(name="outp", bufs=1) as outp,
        ):
            xt = const.tile([128, EXT_COLS], mybir.dt.bfloat16)
            # chunked input on both HWDGE rings (SP + ACT issue in parallel):
            # [AB_r | fb0 plane r] per r, then fb1..fb3 in half-blocks
            engs = [nc.sync, nc.scalar]
            for r in range(4):
                lo = r * CH_COLS
                engs[r % 2].dma_start(xt[:, lo:lo + CH_COLS],
                                      ext[:, lo:lo + CH_COLS])
            half = FB_COLS // 2
            for fb in range(1, N_FB):
                lo = FB1_LO + (fb - 1) * FB_COLS
                engs[fb % 2].dma_start(xt[:, lo:lo + half],
                                       ext[:, lo:lo + half])
                engs[(fb + 1) % 2].dma_start(xt[:, lo + half:lo + FB_COLS],
                                             ext[:, lo + half:lo + FB_COLS])

            # PE preheat: junk matmuls on raw (uninitialized, untracked) SBUF
            # keep the PE busy from the first post-preamble cycle, so HAM is
            # at full clock when the real matmuls start
            junk = nc.alloc_sbuf_tensor("junk", [128, 512],
                                        mybir.dt.bfloat16).ap()
            for _ in range(11):
                ps_w = psum.tile([128, 512], mybir.dt.float32, tag="ps")
                nc.tensor.matmul(ps_w[:], junk[:, :128], junk[:],
                                 start=True, stop=True, skip_group_check=True)

            o = outp.tile([128, N_FC, N_BINS], mybir.dt.float32)

            def mm(ps, fc, r_, a_, start, stop):
                fb, fi = divmod(fc, 4)  # frame block, 128-frame chunk within
                if fb == 0:
                    lo = r_ * CH_COLS + AB_R_COLS + fi * 128 + a_
                else:
                    lo = (FB1_LO + (fb - 1) * FB_COLS + r_ * PLANE_COLS
                          + fi * 128 + a_)
                lhs = xt[:, lo:lo + 128]              # x frames as weights
                rhs = xt[:, r_ * CH_COLS + a_ * 2 * N_BINS:
                         r_ * CH_COLS + (a_ + 1) * 2 * N_BINS]
                nc.tensor.matmul(ps[:], lhs, rhs, start=start, stop=stop)

            def magnitude(ps, fc):
                # a^2 + b^2: squares on ScalarE (parallel to DVE), add on DVE
                sq = tmp.tile([128, 2 * N_BINS], mybir.dt.float32, tag="sq")
                nc.scalar.square(sq[:, :N_BINS], ps[:, :N_BINS])
                nc.scalar.square(sq[:, N_BINS:], ps[:, N_BINS:])
                nc.vector.tensor_add(o[:, fc, :N_BINS],
                                     sq[:, :N_BINS], sq[:, N_BINS:])
                if fc % 4 == 3:
                    g = fc - 3
                    nc.sync.dma_start(
                        out[:, g * N_BINS:(fc + 1) * N_BINS],
                        o[:, g:fc + 1, :].rearrange("p a b -> p (a b)"))

            # fc0-3 interleaved r-major so each input chunk unlocks 16
            # matmuls (matches chunk arrival rate); fc4+ per-fc
            ps03 = [psum.tile([128, 2 * N_BINS], mybir.dt.float32, tag="ps",
                              name=f"ps03_{i}")
                    for i in range(4)]
            for r_ in range(4):
                for fc in range(4):
                    for a_ in range(4):
                        mm(ps03[fc], fc, r_, a_,
                           start=(r_ == 0 and a_ == 0),
                           stop=(r_ == 3 and a_ == 3))
            for fc in range(4):
                magnitude(ps03[fc], fc)
            for fc in range(4, N_FC):
                ps = psum.tile([128, 2 * N_BINS], mybir.dt.float32, tag="ps")
                for i, (r_, a_) in enumerate(
                        (r_, a_) for r_ in range(4) for a_ in range(4)):
                    mm(ps, fc, r_, a_, start=(i == 0), stop=(i == N_KC - 1))
                magnitude(ps, fc)

    _thin_pe_incs(nc, mybir)
    _split_multi_waits(nc, mybir)
    return nc


def _get_program():
    global _PROGRAM
    if _PROGRAM is None:
        _PROGRAM = _build_program()
    return _PROGRAM


def _host_prep(x, wcos, wsin, kr, ki):
    """Fold the CQT kernels; shard, cast, and lay out the waveform."""
    kr64 = np.asarray(kr, dtype=np.float64)
    ki64 = np.asarray(ki, dtype=np.float64)
    wc64 = np.asarray(wcos, dtype=np.float64)
    ws64 = np.asarray(wsin, dtype=np.float64)
    a = kr64 @ wc64 - ki64 @ ws64            # [84, 2048]
    b = kr64 @ ws64 + ki64 @ wc64            # [84, 2048]
    abt = np.concatenate([a, b], axis=0).T   # [2048, 168]
    # abkc[kc][p, j] = abt[kc*128+p, j]
    abkc = abt.reshape(N_KC, 128, 2 * N_BINS).astype(ml_dtypes.bfloat16)

    x = np.asarray(x, dtype=np.float32)
    x_pad = np.zeros((N_CORES - 1) * CORE_STRIDE + SHARD_LEN, dtype=np.float32)
    x_pad[:T_SAMPLES] = x
    x_bf = x_pad.astype(ml_dtypes.bfloat16)
    exts = []
    for c in range(N_CORES):
        shard = x_bf[c * CORE_STRIDE: c * CORE_STRIDE + SHARD_LEN]
        # zz[j, r, p] = x[(4j+r)*128 + p]
        zz = shard.reshape(X_COLS_TOTAL // 4, 4, 128)
        ext = np.empty((128, EXT_COLS), dtype=ml_dtypes.bfloat16)
        for r in range(4):
            lo = r * CH_COLS
            for a_ in range(4):
                ext[:, lo + a_ * 2 * N_BINS: lo + (a_ + 1) * 2 * N_BINS] = (
                    abkc[4 * a_ + r])
            ext[:, lo + AB_R_COLS: lo + CH_COLS] = zz[:PLANE_COLS, r, :].T
        for fb in range(1, N_FB):
            lo = FB1_LO + (fb - 1) * FB_COLS
            for r in range(4):
                ext[:, lo + r * PLANE_COLS: lo + (r + 1) * PLANE_COLS] = (
                    zz[fb * 512: fb * 512 + PLANE_COLS, r, :].T)
        exts.append(ext)
    return exts


_LAST_RESULTS = None  # BassKernelResults of the most recent run (for profiling)


def _ensure_ntff_hook():
    """The image's antenv lacks axon_hooks; recreate it from trn_agent_boot so
    a BASS_TRACE env (set by us or a harness) can't crash the import inside
    run_bass_kernel_spmd."""
    import types

    try:
        import antenv.axon_hooks  # noqa: F401
        return
    except ImportError:
        pass
    try:
        if "/root/.axon_site" not in sys.path:
            sys.path.insert(0, "/root/.axon_site")
        from trn_agent_boot.trn_boot import _ntff_profile_via_ctypes

        hook = _ntff_profile_via_ctypes("/opt/axon/libaxon_pjrt.so")
    except Exception:
        hook = None
    try:
        import antenv

        mod = types.ModuleType("antenv.axon_hooks")
        mod._hook = hook
        mod.get_axon_ntff_profile_hook = lambda: mod._hook
        mod.set_axon_ntff_profile_hook = lambda h: setattr(mod, "_hook", h)
        antenv.axon_hooks = mod
        sys.modules["antenv.axon_hooks"] = mod
    except Exception:
        pass


def kernel(x, wcos, wsin, kr, ki):
    global _LAST_RESULTS
    _ensure_ntff_hook()
    from concourse.bass_utils import run_bass_kernel_spmd

    exts = _host_prep(x, wcos, wsin, kr, ki)
    nc = _get_program()
    in_maps = [{"ext": exts[c]} for c in range(N_CORES)]
    res = run_bass_kernel_spmd(nc, in_maps, core_ids=list(range(N_CORES)))
    _LAST_RESULTS = res
    # per core: out[p, fc*84+j] -> [84, 2048 frames] with frame = fc*128+p
    parts = []
    for c in range(N_CORES):
        oc = res.results[c]["out"].reshape(128, N_FC, N_BINS)
        parts.append(oc.transpose(2, 1, 0).reshape(N_BINS, F_PER_CORE))
    full = np.concatenate(parts, axis=1)
    return np.sqrt(full[None, :, :N_FRAMES]).astype(np.float32)


# revision 47
# speedup vs baseline: 1.0602x; 1.0602x over previous
"""Trainium2 Bass kernel: CQT (constant-Q transform) of 2^23 audio samples.

Reference math (jax):
    frames[f, n] = x[f*HOP + n]                  HOP=512, fftLen=2048
    four_r = frames @ wcos.T ; four_i = frames @ wsin.T
    cqt_r  = kr @ four_r - ki @ four_i
    cqt_i  = kr @ four_i + ki @ four_r
    out    = sqrt(cqt_r**2 + cqt_i**2)           # [1, 84, n_frames]

Folded on the host (exact algebra, tiny matrices):
    A = kr@wcos - ki@wsin,  B = kr@wsin + ki@wcos      (each [84, 2048])
    out = sqrt((A @ frames.T)**2 + (B @ frames.T)**2)

Device strategy (8-way shard along the frame axis; kernels replicated):
  - 2048 frames per core.  The bf16 x-shard is laid out host-side so every
    matmul operand is a CONTIGUOUS column range: with xt[p, c] =
    x[c*128 + p], contraction chunk kc = 4a + r of frame f needs column
    4*(f+a) + r, so columns are stored deinterleaved by (frame-block,
    r-plane).  A.T/B.T chunks ride the same DRAM tensor.  (A strided AP
    halves the PE's bf16 stream rate - measured 452 vs 216 ns per matmul.)
  - matmuls run "orientation 2": 128-frame x-chunks are the stationary
    operand (full 128 PE columns, FWL-eligible), the [A|B] chunk [128, 168]
    streams; 16 frame-chunks x 16 K-chunks accumulate in PSUM; squares on
    ScalarE + add on VectorE; outputs leave frames-major per 4 chunks.
  - overlap: chunked input DMAs on both HWDGE rings; junk matmuls on raw
    SBUF preheat the PE clock (HAM) while the first chunk lands; fc0-3 run
    r-major interleaved to match chunk arrival.  sqrt on the host.
  - post-passes for this toolchain: multi-wait instructions are split onto
    injected NoOps (walrus encodes at most ONE sem wait per instruction),
    non-group-end matmul PE-sem increments are stripped (PE sequencer
    retires incs at ~115ns), and the Tile entry/exit all-engine barriers
    are elided (single-shot NEFF; the SP drain still waits every proc).
"""

import sys

if "/opt/trn_rl_repo" not in sys.path:
    sys.path.insert(0, "/opt/trn_rl_repo")

import numpy as np
import ml_dtypes

HOP = 512
FFTLEN = 2048
N_BINS = 84
T_SAMPLES = 8388608
N_FRAMES = (T_SAMPLES - FFTLEN) // HOP + 1  # 16381
N_CORES = 8
F_PER_CORE = 2048                 # frames computed per core (3 junk at the end)
X_COLS_TOTAL = 8204               # sample columns actually needed per core
SHARD_LEN = X_COLS_TOTAL * 128    # 1050112 samples per core
CORE_STRIDE = F_PER_CORE * HOP    # 1048576 samples between shard starts
N_KC = FFTLEN // 128              # 16 contraction chunks
N_FB = F_PER_CORE // 512          # 4 frame blocks of 512 frames
PLANE_COLS = 515                  # columns per r-plane per frame block
FB_COLS = 4 * PLANE_COLS          # 2060
AB_R_COLS = 4 * 2 * N_BINS        # 672: the 4 kc-chunks of A.T/B.T for one r
CH_COLS = AB_R_COLS + PLANE_COLS  # 1187: one [AB_r | fb0 plane r] chunk
FB1_LO = 4 * CH_COLS              # 4748: start of the fb1..fb3 blocks
AB_COLS = N_KC * 2 * N_BINS       # 2688 columns holding A.T/B.T chunks
EXT_COLS = FB1_LO + (N_FB - 1) * FB_COLS  # 10928
N_FC = F_PER_CORE // 128          # 16 output frame chunks (128 frames each)

_PROGRAM = None


def _thin_pe_incs(nc, mybir):
    """Matmuls complete in pc order, so only each accumulation group's last
    matmul needs its PE-semaphore increment.  The PE sequencer retires incs
    at ~115ns each - 267 of them add ~6us of pure sem-retirement tail.
    Strip non-stop matmul incs and renumber every wait on that semaphore."""
    sem_id = None
    tick = 0
    kept = 0
    tick_to_kept = {0: 0}
    for f in nc.m.functions:
        for blk in f.blocks:
            for inst in blk.instructions:
                si = getattr(inst, "sync_info", None)
                if si is None:
                    continue
                pe_ups = [u for u in si.on_update
                          if u.ant_name.startswith("PE")]
                if not pe_ups:
                    continue
                if type(inst).__name__ != "InstMatmult":
                    return  # unexpected PE-sem producer; skip optimization
                sem_id = pe_ups[0].id
                tick += 1
                if inst.stop_tensor_calc:
                    kept += 1
                else:
                    inst.sync_info = mybir.SyncInfo(
                        on_wait=list(si.on_wait),
                        on_update=[u for u in si.on_update
                                   if not u.ant_name.startswith("PE")])
                tick_to_kept[tick] = kept
    if sem_id is None:
        return
    for f in nc.m.functions:
        for blk in f.blocks:
            for inst in blk.instructions:
                si = getattr(inst, "sync_info", None)
                if si is None:
                    continue
                changed = False
                new_waits = []
                for w in si.on_wait:
                    if w.id == sem_id and w.wait_value in tick_to_kept:
                        nv = tick_to_kept[w.wait_value]
                        if nv != w.wait_value:
                            w = mybir.SyncWait(
                                sync_type=w.sync_type, id=w.id,
                                ant_name=w.ant_name, wait_mode=w.wait_mode,
                                wait_value=nv, wait_reg=w.wait_reg)
                            changed = True
                    new_waits.append(w)
                if changed:
                    inst.sync_info = mybir.SyncInfo(
                        on_wait=new_waits, on_update=list(si.on_update))


def _split_multi_waits(nc, mybir, max_waits=1):
    """This walrus build encodes at most one sem wait per instruction; move
    extra waits onto injected same-engine NoOps right before the instruction."""
    ctr = 0
    for f in nc.m.functions:
        for blk in f.blocks:
            il = list(blk.instructions)
            new = []
            changed = False
            for inst in il:
                si = getattr(inst, "sync_info", None)
                if si is not None and len(si.on_wait) > max_waits:
                    waits = list(si.on_wait)
                    for w in waits[:-max_waits]:
                        nop = mybir.InstNoOp(name=f"I-waitfix-{ctr}", ins=[], outs=[])
                        ctr += 1
                        nop.engine = inst.engine
                        nop.sync_info = mybir.SyncInfo(on_wait=[w], on_update=[])
                        new.append(nop)
                    inst.sync_info = mybir.SyncInfo(
                        on_wait=waits[-max_waits:], on_update=list(si.on_update))
                    changed = True
                new.append(inst)
            if changed:
                blk.instructions = new


def _build_program():
    import concourse.bass as bass
    import concourse.tile as tile
    from concourse import mybir
    from concourse.vector_clock import ScopedClock

    def _lean_drain(self, tick_clock, wait_clock):
        # Tail for a single-shot NEFF: the SP drain already waits on every
        # proc's final tick (incl. output-DMA completion).  The stock
        # drain+barrier+sem-reset+barrier tail costs ~7us and only matters
        # for re-executing a loaded NEFF with dirty semaphores.
        drain_inst = self.nc.sync.drain()
        wait_clock.add_sem_waits(
            drain_inst.ins, ScopedClock({None: tick_clock.global_clock}))
        popped = self.nc._tile_sem_poison_stack.pop()
        assert popped is self._sem_poison

    tile.TileContext._drain_and_barrier = _lean_drain

    # Skip the ~3.4us entry all-engine barrier: it orders the preamble's
    # const-AP writes (PE, t~0.4us) and SWDGE scratch memsets against the
    # body.  This kernel reads const APs first at ~13us (ACT square bias)
    # and issues no SWDGE DMAs, so engine start-skew cannot race it.
    _orig_barrier = bass.Bass.all_engine_barrier
    bass.Bass.all_engine_barrier = lambda self, **kw: None
    try:
        nc = bass.Bass("TRN2", target_bir_lowering=False, debug=False)
    finally:
        bass.Bass.all_engine_barrier = _orig_barrier

    ext = nc.dram_tensor("ext", [128, EXT_COLS], mybir.dt.bfloat16,
                         kind="ExternalInput").ap()
    # out[p, fc*84+j] = |cqt|^2 at frame fc*128+p, bin j
    out = nc.dram_tensor("out", [128, N_FC * N_BINS], mybir.dt.float32,
                         kind="ExternalOutput").ap()

    with tile.TileContext(nc) as tc:
        with (
            tc.tile_pool(name="const", bufs=1) as const,
            tc.tile_pool(name="psum", bufs=4, space="PSUM") as psum,
            tc.tile_pool(name="tmp", bufs=4) as tmp,
            tc.tile_pool(name="outp", bufs=1) as outp,
        ):
            xt = const.tile([128, EXT_COLS], mybir.dt.bfloat16)
            # chunked input on both HWDGE rings (SP + ACT issue in parallel):
            # [AB_r | fb0 plane r] per r, then fb1..fb3 in half-blocks
            engs = [nc.sync, nc.scalar]
            for r in range(4):
                lo = r * CH_COLS
                engs[r % 2].dma_start(xt[:, lo:lo + CH_COLS],
                                      ext[:, lo:lo + CH_COLS])
            half = FB_COLS // 2
            for fb in range(1, N_FB):
                lo = FB1_LO + (fb - 1) * FB_COLS
                engs[fb % 2].dma_start(xt[:, lo:lo + half],
                                       ext[:, lo:lo + half])
                engs[(fb + 1) % 2].dma_start(xt[:, lo + half:lo + FB_COLS],
                                             ext[:, lo + half:lo + FB_COLS])

            # PE preheat: junk matmuls on raw (uninitialized, untracked) SBUF
            # keep the PE busy from the first post-preamble cycle, so HAM is
            # at full clock when the real matmuls start
            junk = nc.alloc_sbuf_tensor("junk", [128, 512],
                                        mybir.dt.bfloat16).ap()
            for _ in range(11):
                ps_w = psum.tile([128, 512], mybir.dt.float32, tag="ps")
                nc.tensor.matmul(ps_w[:], junk[:, :128], junk[:],
                                 start=True, stop=True, skip_group_check=True)

            o = outp.tile([128, N_FC, N_BINS], mybir.dt.float32)

            def mm(ps, fc, r_, a_, start, stop):
                fb, fi = divmod(fc, 4)  # frame block, 128-frame chunk within
                if fb == 0:
                    lo = r_ * CH_COLS + AB_R_COLS + fi * 128 + a_
                else:
                    lo = (FB1_LO + (fb - 1) * FB_COLS + r_ * PLANE_COLS
                          + fi * 128 + a_)
                lhs = xt[:, lo:lo + 128]              # x frames as weights
                rhs = xt[:, r_ * CH_COLS + a_ * 2 * N_BINS:
                         r_ * CH_COLS + (a_ + 1) * 2 * N_BINS]
                nc.tensor.matmul(ps[:], lhs, rhs, start=start, stop=stop)

            def magnitude(ps, fc):
                # a^2 + b^2: squares on ScalarE (parallel to DVE), add on DVE
                sq = tmp.tile([128, 2 * N_BINS], mybir.dt.float32, tag="sq")
                nc.scalar.square(sq[:, :N_BINS], ps[:, :N_BINS])
                nc.scalar.square(sq[:, N_BINS:], ps[:, N_BINS:])
                nc.vector.tensor_add(o[:, fc, :N_BINS],
                                     sq[:, :N_BINS], sq[:, N_BINS:])
                if fc % 4 == 3:
                    g = fc - 3
                    nc.sync.dma_start(
                        out[:, g * N_BINS:(fc + 1) * N_BINS],
                        o[:, g:fc + 1, :].rearrange("p a b -> p (a b)"))

            # fc0-3 interleaved r-major so each input chunk unlocks 16
            # matmuls (matches chunk arrival rate); fc4+ per-fc
            ps03 = [psum.tile([128, 2 * N_BINS], mybir.dt.float32, tag="ps",
                              name=f"ps03_{i}")
                    for i in range(4)]
            for r_ in range(4):
                for fc in range(4):
                    for a_ in range(4):
                        mm(ps03[fc], fc, r_, a_,
                           start=(r_ == 0 and a_ == 0),
                           stop=(r_ == 3 and a_ == 3))
            for fc in range(4):
                magnitude(ps03[fc], fc)
            for fc in range(4, N_FC):
                ps = psum.tile([128, 2 * N_BINS], mybir.dt.float32, tag="ps")
                for i, (r_, a_) in enumerate(
                        (r_, a_) for r_ in range(4) for a_ in range(4)):
                    mm(ps, fc, r_, a_, start=(i == 0), stop=(i == N_KC - 1))
                magnitude(ps, fc)

    _thin_pe_incs(nc, mybir)
    _split_multi_waits(nc, mybir)
    return nc


def _get_program():
    global _PROGRAM
    if _PROGRAM is None:
        _PROGRAM = _build_program()
    return _PROGRAM


def _host_prep(x, wcos, wsin, kr, ki):
    """Fold the CQT kernels; shard, cast, and lay out the waveform."""
    kr64 = np.asarray(kr, dtype=np.float64)
    ki64 = np.asarray(ki, dtype=np.float64)
    wc64 = np.asarray(wcos, dtype=np.float64)
    ws64 = np.asarray(wsin, dtype=np.float64)
    a = kr64 @ wc64 - ki64 @ ws64            # [84, 2048]
    b = kr64 @ ws64 + ki64 @ wc64            # [84, 2048]
    abt = np.concatenate([a, b], axis=0).T   # [2048, 168]
    # abkc[kc][p, j] = abt[kc*128+p, j]
    abkc = abt.reshape(N_KC, 128, 2 * N_BINS).astype(ml_dtypes.bfloat16)

    x = np.asarray(x, dtype=np.float32)
    x_pad = np.zeros((N_CORES - 1) * CORE_STRIDE + SHARD_LEN, dtype=np.float32)
    x_pad[:T_SAMPLES] = x
    x_bf = x_pad.astype(ml_dtypes.bfloat16)
    exts = []
    for c in range(N_CORES):
        shard = x_bf[c * CORE_STRIDE: c * CORE_STRIDE + SHARD_LEN]
        # zz[j, r, p] = x[(4j+r)*128 + p]
        zz = shard.reshape(X_COLS_TOTAL // 4, 4, 128)
        ext = np.empty((128, EXT_COLS), dtype=ml_dtypes.bfloat16)
        for r in range(4):
            lo = r * CH_COLS
            for a_ in range(4):
                ext[:, lo + a_ * 2 * N_BINS: lo + (a_ + 1) * 2 * N_BINS] = (
                    abkc[4 * a_ + r])
            ext[:, lo + AB_R_COLS: lo + CH_COLS] = zz[:PLANE_COLS, r, :].T
        for fb in range(1, N_FB):
            lo = FB1_LO + (fb - 1) * FB_COLS
            for r in range(4):
                ext[:, lo + r * PLANE_COLS: lo + (r + 1) * PLANE_COLS] = (
                    zz[fb * 512: fb * 512 + PLANE_COLS, r, :].T)
        exts.append(ext)
    return exts


_LAST_RESULTS = None  # BassKernelResults of the most recent run (for profiling)


def _ensure_ntff_hook():
    """The image's antenv lacks axon_hooks; recreate it from trn_agent_boot so
    a BASS_TRACE env (set by us or a harness) can't crash the import inside
    run_bass_kernel_spmd."""
    import types

    try:
        import antenv.axon_hooks  # noqa: F401
        return
    except ImportError:
        pass
    try:
        if "/root/.axon_site" not in sys.path:
            sys.path.insert(0, "/root/.axon_site")
        from trn_agent_boot.trn_boot import _ntff_profile_via_ctypes

        hook = _ntff_profile_via_ctypes("/opt/axon/libaxon_pjrt.so")
    except Exception:
        hook = None
    try:
        import antenv

        mod = types.ModuleType("antenv.axon_hooks")
        mod._hook = hook
        mod.get_axon_ntff_profile_hook = lambda: mod._hook
        mod.set_axon_ntff_profile_hook = lambda h: setattr(mod, "_hook", h)
        antenv.axon_hooks = mod
        sys.modules["antenv.axon_hooks"] = mod
    except Exception:
        pass


def kernel(x, wcos, wsin, kr, ki):
    global _LAST_RESULTS
    _ensure_ntff_hook()
    from concourse.bass_utils import run_bass_kernel_spmd

    exts = _host_prep(x, wcos, wsin, kr, ki)
    nc = _get_program()
    in_maps = [{"ext": exts[c]} for c in range(N_CORES)]
    res = run_bass_kernel_spmd(nc, in_maps, core_ids=list(range(N_CORES)))
    _LAST_RESULTS = res
    # per core: out[p, fc*84+j] -> [84, 2048 frames] with frame = fc*128+p
    parts = []
    for c in range(N_CORES):
        oc = res.results[c]["out"].reshape(128, N_FC, N_BINS)
        parts.append(oc.transpose(2, 1, 0).reshape(N_BINS, F_PER_CORE))
    full = np.concatenate(parts, axis=1)
    return np.sqrt(full[None, :, :N_FRAMES]).astype(np.float32)
